# revision 43
# baseline (speedup 1.0000x reference)
"""Trainium2 Bass kernel for a 2-layer autoregressive LSTM LM.

Model: B=512, T=128, V=36, E=128, H=512.
  emb = emb_table[x]                                   [B, T, E]
  2-layer LSTM scan over T-1 steps (PyTorch gate order i,f,g,o)
  logits_t = relu(h1_t @ W_fc.T + b_fc)                [B, V]
  probs_t  = softmax(logits_t)
  nll_t    = -logits_t[n, tgt] masked at tgt==ignore(0)
  sample_loss = sum_t nll_t / count(x != 0)
Returns (probs [B,T-1,V], emb [B,T,E], sample_loss [B], mean_loss scalar).

Sharding: data-parallel over batch, 64 rows per core on 8 NeuronCores.
No collectives needed; mean_loss is reduced at host gather time.

Device-side structure (per core, single NEFF):
  prep:  U = emb_table @ W_ih0.T + (b_ih0+b_hh0)   [36, 2048] in SBUF
         ohT[v, t*64+b] = (x[b,t] == v)            one-hot.T via K=1
              broadcast-matmul + is_equal, for all 127 steps
         emb output rows gathered by indirect DMA from emb_table
  scan:  per step t (fully unrolled, batch on PSUM partitions M=64):
         g0 = ohT_t.T @ U + h0.T.T @ W_hh0.T       (PSUM accum, K=36+4x128)
         g1 = ones.T @ (b_ih1+b_hh1) + h0 @ W_ih1.T + h1 @ W_hh1.T
         gate nonlinearities on ACT straight from PSUM, c/h updates on DVE,
         h -> h.T via PE transposes (identity matmul) for the next step's
         stationary operand, fc/softmax/nll inline.
  All matmuls run as float32r (full-rate fp32 PE mode).
"""

import os
import numpy as np
from contextlib import ExitStack

import concourse.bass as bass
import concourse.mybir as mybir
import concourse.tile as tile
from concourse import bacc
from concourse.bass import IndirectOffsetOnAxis
from concourse.bass_utils import run_bass_kernel_spmd
from concourse.masks import make_identity

AF = mybir.ActivationFunctionType
ALU = mybir.AluOpType
DT = mybir.dt

B, T, V, E, H = 512, 128, 36, 128, 512
NCORES = 8
BL = B // NCORES          # 64 batch rows per core
G = 4 * H                 # 2048 gate width
NB = G // 512             # 4 psum-bank-wide gate chunks
KC = H // 128             # 4 K-chunks of the hidden dim

MM_DT = DT.float32r       # PE compute dtype (full-rate fp32 mode)


def _r_unused(ap):
    """Bitcast an f32 AP to the PE fast-fp32 dtype."""
    return ap.bitcast(MM_DT)


def build_graph(t_steps=T):
    """Build the SPMD single-core graph. t_steps is the number of x columns
    used (T in production; smaller for simulator smoke tests)."""
    ts_ = t_steps
    nsteps = ts_ - 1

    nc = bacc.Bacc(None, target_bir_lowering=False)

    # ---- DRAM parameters (per-core shards / replicated weights) ----
    p_x = nc.declare_dram_parameter("x_i", [BL, ts_], DT.int32, isOutput=False)
    # x[:, :nsteps] flattened t-major: [1, nsteps*BL]
    p_xrow = nc.declare_dram_parameter("x_row", [1, nsteps * BL], DT.int32, isOutput=False)
    p_embT = nc.declare_dram_parameter("embT", [E, V], DT.float32, isOutput=False)
    p_emb = nc.declare_dram_parameter("emb_tab", [V, E], DT.float32, isOutput=False)
    p_wih0T = nc.declare_dram_parameter("wih0T", [E, G], DT.float32, isOutput=False)
    p_wh0T = nc.declare_dram_parameter("wh0T", [H, G], DT.float32, isOutput=False)
    p_wi1T = nc.declare_dram_parameter("wi1T", [H, G], DT.float32, isOutput=False)
    p_wh1T = nc.declare_dram_parameter("wh1T", [H, G], DT.float32, isOutput=False)
    p_wfcT = nc.declare_dram_parameter("wfcT", [H, V], DT.float32, isOutput=False)
    p_bi0 = nc.declare_dram_parameter("b_i0", [1, G], DT.float32, isOutput=False)
    p_bh0 = nc.declare_dram_parameter("b_h0", [1, G], DT.float32, isOutput=False)
    p_bi1 = nc.declare_dram_parameter("b_i1", [1, G], DT.float32, isOutput=False)
    p_bh1 = nc.declare_dram_parameter("b_h1", [1, G], DT.float32, isOutput=False)
    p_bfc = nc.declare_dram_parameter("b_fc", [1, V], DT.float32, isOutput=False)

    o_probs = nc.declare_dram_parameter("probs", [BL, nsteps, V], DT.float32, isOutput=True)
    o_emb = nc.declare_dram_parameter("emb", [BL, ts_, E], DT.float32, isOutput=True)
    o_sloss = nc.declare_dram_parameter("sloss", [BL, 1], DT.float32, isOutput=True)

    with ExitStack() as ctx:
        tc = ctx.enter_context(tile.TileContext(nc))
        # persistent tensors (live for the whole kernel)
        wpool = ctx.enter_context(tc.tile_pool(name="wts", bufs=1))
        epool = ctx.enter_context(tc.tile_pool(name="embg", bufs=2))

        # ---------------- persistent constants + weights ----------------
        x_int = wpool.tile([BL, ts_], DT.int32, tag="x_int")
        nc.sync.dma_start(x_int[:], p_x[:])
        xf = wpool.tile([BL, ts_], DT.float32, tag="xf")
        nc.vector.tensor_copy(xf[:], x_int[:])

        ident = wpool.tile([BL, BL], DT.float32, tag="ident")
        make_identity(nc, ident[:])

        iota_i = wpool.tile([BL, V], DT.int32, tag="iota_i")
        nc.gpsimd.iota(iota_i[:], pattern=[[1, V]], base=0, channel_multiplier=0)
        iota36 = wpool.tile([BL, V], DT.float32, tag="iota36")
        nc.vector.tensor_copy(iota36[:], iota_i[:])

        ones_f = wpool.tile([1, BL], DT.float32, tag="ones_f")
        nc.gpsimd.memset(ones_f[:], 1.0)
        ones_b = wpool.tile([1, BL], MM_DT, tag="ones_b")
        nc.vector.tensor_copy(ones_b[:], ones_f[:])
        ones_v = wpool.tile([1, V], MM_DT, tag="ones_v")
        nc.vector.tensor_copy(ones_v[:], ones_f[:, :V])

        # weights to SBUF; K-chunk k of W.T lives at cols [k*G, (k+1)*G)
        # (weight DMAs are emitted in the prep scope after the prep-critical
        # small loads so the U/ohT builds are not queued behind 13 MB)
        w_hh0 = wpool.tile([128, KC * G], MM_DT, tag="w_hh0")
        w_ih1 = wpool.tile([128, KC * G], MM_DT, tag="w_ih1")
        w_hh1 = wpool.tile([128, KC * G], MM_DT, tag="w_hh1")
        w_fc = wpool.tile([128, KC * V], MM_DT, tag="w_fc")

        bfc_sb = wpool.tile([1, V], MM_DT, tag="bfc_sb")
        nc.gpsimd.dma_start(bfc_sb[:], p_bfc[:])

        bb = wpool.tile([BL, G], DT.bfloat16, tag="bb")
        bfb = wpool.tile([BL, V], DT.float32, tag="bfb")
        lgbuf = wpool.tile([BL, max(nsteps, 2) * V], DT.bfloat16, tag="lgbuf")
        u_sb = wpool.tile([BL, G], MM_DT, tag="u_sb")
        ohT = wpool.tile([BL, nsteps * BL], MM_DT, tag="ohT")
        npad = wpool.tile([BL, ts_], DT.float32, tag="npad")
        invlen = wpool.tile([BL, 1], DT.float32, tag="invlen")
        nllb = wpool.tile([BL, max(nsteps, 2)], DT.float32, tag="nllb")

        # ---------------- prep scope (released before the scan) ----------------
        with tc.tile_pool(name="prep", bufs=2) as ppool, \
             tc.tile_pool(name="ppsum", bufs=2, space="PSUM") as ppsum:
            zrow = ppool.tile([32, G], DT.float32, tag="btmp", bufs=1)
            nc.gpsimd.memset(zrow[:], 0.0)
            nc.scalar.copy(u_sb[32:BL, :], zrow[:])
            iotc_i = ppool.tile([BL, 1], DT.int32, tag="iotc_i", bufs=1)
            nc.gpsimd.iota(iotc_i[:], pattern=[[1, 1]], base=0, channel_multiplier=1)
            iotc = ppool.tile([BL, 1], DT.float32, tag="iotc", bufs=1)
            nc.vector.tensor_copy(iotc[:], iotc_i[:])

            embT_sb = ppool.tile([E, V], MM_DT, tag="embT_sb", bufs=1)
            nc.gpsimd.dma_start(embT_sb[:], p_embT[:])
            wih0_sb = ppool.tile([E, G], MM_DT, tag="wih0_sb", bufs=1)
            nc.gpsimd.dma_start(wih0_sb[:], p_wih0T[:])

            # bias sums BEFORE the big weight DMAs on the gpsimd queue, so the
            # U/bias-broadcast builds are not stalled behind 13 MB of weights
            b0s = ppool.tile([1, G], MM_DT, tag="b0s", bufs=1)
            nc.gpsimd.dma_start(b0s[:], p_bi0[:])
            bt = ppool.tile([1, G], DT.float32, tag="btmp", bufs=1)
            nc.sync.dma_start(bt[:], p_bh0[:])
            nc.vector.tensor_add(b0s[:], b0s[:], bt[:])
            b1s = ppool.tile([1, G], MM_DT, tag="b1s", bufs=1)
            nc.gpsimd.dma_start(b1s[:], p_bi1[:])
            bt2 = ppool.tile([1, G], DT.float32, tag="btmp", bufs=1)
            nc.sync.dma_start(bt2[:], p_bh1[:])
            nc.vector.tensor_add(b1s[:], b1s[:], bt2[:])

            for k in range(KC):
                nc.gpsimd.dma_start(w_hh0[:, k * G:(k + 1) * G], p_wh0T[k * 128:(k + 1) * 128, :])
                nc.gpsimd.dma_start(w_ih1[:, k * G:(k + 1) * G], p_wi1T[k * 128:(k + 1) * 128, :])
                nc.gpsimd.dma_start(w_hh1[:, k * G:(k + 1) * G], p_wh1T[k * 128:(k + 1) * 128, :])
            for k in range(KC):
                nc.gpsimd.dma_start(w_fc[:, k * V:(k + 1) * V], p_wfcT[k * 128:(k + 1) * 128, :])

            # U = emb_table @ W_ih0.T + b0s : [V, G] (rows V..BL stay zero)
            for n in range(NB):
                ps = ppsum.tile([V, 512], DT.float32, tag="pp", name=f"ups_{n}")
                nc.tensor.matmul(ps[:], (embT_sb[:]), (wih0_sb[:, n * 512:(n + 1) * 512]),
                                 start=True, stop=False)
                nc.tensor.matmul(ps[:], (ones_v[:]), (b0s[:, n * 512:(n + 1) * 512]),
                                 start=False, stop=True)
                nc.scalar.copy(u_sb[0:V, n * 512:(n + 1) * 512], ps[:])

            # ohT[v, t*BL+b] = (x[b, t] == v) for the input steps
            ncols = nsteps * BL
            pos = 0
            ci = 0
            while pos < ncols:
                w = min(512, ncols - pos)
                xr_i = ppool.tile([1, 512], DT.int32, tag="xr_i", bufs=4, name=f"xri_{ci}")
                nc.sync.dma_start(xr_i[:, :w], p_xrow[:, pos:pos + w])
                xr_f = ppool.tile([1, 512], MM_DT, tag="xr_f", bufs=4, name=f"xrf_{ci}")
                nc.vector.tensor_copy(xr_f[:, :w], xr_i[:, :w])
                ps = ppsum.tile([BL, 512], DT.float32, tag="pp", name=f"ohps_{ci}")
                nc.tensor.matmul(ps[:, :w], (ones_b[:]), (xr_f[:, :w]),
                                 start=True, stop=True)
                nc.vector.tensor_scalar(ohT[:, pos:pos + w], ps[:, :w], iotc[:, 0:1], None,
                                        op0=ALU.is_equal)
                pos += w
                ci += 1

            # broadcast fc bias to all batch partitions
            psb = ppsum.tile([BL, V], DT.float32, tag="ppb", bufs=2)
            nc.tensor.matmul(psb[:], (ones_b[:]), (bfc_sb[:]), start=True, stop=True)
            nc.scalar.copy(bfb[:], psb[:])

            # broadcast L1 bias to all batch partitions (bf16 is plenty)
            for n in range(NB):
                ps = ppsum.tile([BL, 512], DT.float32, tag="ppb", name=f"bb_{n}")
                nc.tensor.matmul(ps[:], (ones_b[:]), (b1s[:, n * 512:(n + 1) * 512]),
                                 start=True, stop=True)
                nc.scalar.copy(bb[:, n * 512:(n + 1) * 512], ps[:])

            # pad mask + inverse length
            nc.vector.tensor_scalar(npad[:], xf[:], 0.0, None, op0=ALU.not_equal)
            lenr = ppool.tile([BL, 1], DT.float32, tag="lenr", bufs=1)
            nc.vector.reduce_sum(lenr[:], npad[:], axis=mybir.AxisListType.X)
            nc.vector.reciprocal(invlen[:], lenr[:])

            # emb output: gather rows of emb_table by token id, chunked DMA out
            EC = 4  # t's per chunk
            for c0 in range(0, ts_, EC):
                cw = min(EC, ts_ - c0)
                ebuf = epool.tile([BL, EC * E], DT.float32, tag="ebuf", name=f"ebuf_{c0}")
                for j in range(cw):
                    tcol = c0 + j
                    nc.gpsimd.indirect_dma_start(
                        out=ebuf[:, j * E:(j + 1) * E],
                        out_offset=None,
                        in_=p_emb[:],
                        in_offset=IndirectOffsetOnAxis(ap=x_int[:, tcol:tcol + 1], axis=0),
                    )
                nc.sync.dma_start(o_emb[:, c0:c0 + cw, :], ebuf[:, :cw * E])

        # ---------------- scan pools ----------------
        spool = ctx.enter_context(tc.tile_pool(name="work", bufs=2))
        apool = ctx.enter_context(tc.tile_pool(name="acts", bufs=6))
        hpool = ctx.enter_context(tc.tile_pool(name="hT", bufs=4))
        cpool = ctx.enter_context(tc.tile_pool(name="cc", bufs=2))
        gp = ctx.enter_context(tc.tile_pool(name="gpsum", bufs=4, space="PSUM"))
        trp = ctx.enter_context(tc.tile_pool(name="trpsum", bufs=3, space="PSUM"))
        fcp = ctx.enter_context(tc.tile_pool(name="fcpsum", bufs=1, space="PSUM"))

        GFUNC = [AF.Sigmoid, AF.Sigmoid, AF.Tanh, AF.Sigmoid]  # i, f, g, o

        h0T_p = None  # [128, KC*BL] transposed hidden, layer 0, prev step
        h1T_p = None
        c0_p = None
        c1_p = None
        fc_pend = None  # (psum, t) for the deferred fc tail of step t-1

        def emit_fc(fcps, t_):
            # relu on DVE straight into the bf16 logits buffer (no ACT table swap)
            lg = lgbuf[:, t_ * V:(t_ + 1) * V]
            nc.vector.tensor_scalar_max(lg, fcps[:], 0.0)
            # nll: logits[tgt] * (tgt != 0), accumulated for the end
            oht = spool.tile([BL, V], DT.float32, tag="oht", name=f"oht_{t_}")
            nc.vector.tensor_scalar(oht[:], iota36[:], xf[:, t_ + 1:t_ + 2], None,
                                    op0=ALU.is_equal)
            pick = spool.tile([BL, V], DT.float32, tag="pick", name=f"pick_{t_}")
            nc.vector.tensor_mul(pick[:], lg, oht[:])
            nv = spool.tile([BL, 1], DT.float32, tag="nv", name=f"nv_{t_}")
            nc.vector.reduce_sum(nv[:], pick[:], axis=mybir.AxisListType.X)
            nc.vector.tensor_mul(nllb[:, t_:t_ + 1], nv[:], npad[:, t_ + 1:t_ + 2])

        for t in range(nsteps):
            oh_t = ohT[:, t * BL:(t + 1) * BL]

            # --- L0 matmuls: g0 = oh_t.T @ U (+ h0T_p.T @ Whh0.T) ---
            g0 = [gp.tile([BL, 512], DT.float32, tag="g", name=f"g0_{t}_{n}") for n in range(NB)]
            for n in range(NB):
                nc.tensor.matmul(g0[n][:], (oh_t), (u_sb[:, n * 512:(n + 1) * 512]),
                                 start=True, stop=(h0T_p is None))
                if h0T_p is not None:
                    for k in range(KC):
                        nc.tensor.matmul(
                            g0[n][:], (h0T_p[:, k * BL:(k + 1) * BL]),
                            (w_hh0[:, k * G + n * 512:k * G + (n + 1) * 512]),
                            start=False, stop=(k == KC - 1))

            # --- L1 matmuls part 1: bias + Whh1 (uses h1T_p) ---
            g1 = [gp.tile([BL, 512], DT.float32, tag="g", name=f"g1_{t}_{n}") for n in range(NB)]
            if h1T_p is not None:
                for n in range(NB):
                    for k in range(KC):
                        nc.tensor.matmul(
                            g1[n][:], (h1T_p[:, k * BL:(k + 1) * BL]),
                            (w_hh1[:, k * G + n * 512:k * G + (n + 1) * 512]),
                            start=(k == 0), stop=False)

            # --- deferred fc tail of the previous step (keeps PE dense) ---
            if fc_pend is not None:
                emit_fc(*fc_pend)
                fc_pend = None

            # --- L0 elementwise: gates -> c0, h0, h0T ---
            a0 = []
            for n in range(NB):
                an = apool.tile([BL, 512], DT.float32, tag="a", name=f"a0_{t}_{n}")
                nc.scalar.activation(an[:], g0[n][:], GFUNC[n])
                a0.append(an)
            tig = spool.tile([BL, H], DT.float32, tag="tig", name=f"tig0_{t}")
            nc.vector.tensor_mul(tig[:], a0[0][:], a0[2][:])      # i*tanh(g)
            c0 = cpool.tile([BL, H], DT.float32, tag="c0", name=f"c0_{t}")
            if c0_p is None:
                nc.vector.tensor_copy(c0[:], tig[:])
            else:
                nc.vector.tensor_mul(c0[:], a0[1][:], c0_p[:])    # f*c
                nc.vector.tensor_add(c0[:], c0[:], tig[:])
            c0_p = c0
            tch = spool.tile([BL, H], DT.float32, tag="tch", name=f"tch0_{t}")
            nc.scalar.activation(tch[:], c0[:], AF.Tanh)
            h0 = spool.tile([BL, H], DT.float32, tag="h", name=f"h0_{t}")
            nc.vector.tensor_mul(h0[:], a0[3][:], tch[:])         # o*tanh(c)
            h0T = hpool.tile([128, KC * BL], MM_DT, tag="h0T", name=f"h0T_{t}")
            for k in range(KC):
                trt = trp.tile([128, BL], DT.float32, tag="tr", name=f"tr0_{t}_{k}")
                nc.tensor.transpose(trt[:], h0[:, k * 128:(k + 1) * 128], ident[:])
                if k % 2 == 0:
                    nc.vector.tensor_copy(h0T[:, k * BL:(k + 1) * BL], trt[:])
                else:
                    nc.scalar.copy(h0T[:, k * BL:(k + 1) * BL], trt[:])

            # --- L1 matmuls part 2: W_ih1 with fresh h0T (closes accum) ---
            for n in range(NB):
                for k in range(KC):
                    nc.tensor.matmul(
                        g1[n][:], (h0T[:, k * BL:(k + 1) * BL]),
                        (w_ih1[:, k * G + n * 512:k * G + (n + 1) * 512]),
                        start=(h1T_p is None and k == 0), stop=(k == KC - 1))
                # bias add in PSUM (DVE), instead of a K=1 matmul on PE
                nc.vector.tensor_add(g1[n][:], g1[n][:], bb[:, n * 512:(n + 1) * 512])

            # --- L1 elementwise ---
            a1 = []
            for n in range(NB):
                an = apool.tile([BL, 512], DT.float32, tag="a", name=f"a1_{t}_{n}")
                nc.scalar.activation(an[:], g1[n][:], GFUNC[n])
                a1.append(an)
            tig1 = spool.tile([BL, H], DT.float32, tag="tig", name=f"tig1_{t}")
            nc.vector.tensor_mul(tig1[:], a1[0][:], a1[2][:])
            c1 = cpool.tile([BL, H], DT.float32, tag="c1", name=f"c1_{t}")
            if c1_p is None:
                nc.vector.tensor_copy(c1[:], tig1[:])
            else:
                nc.vector.tensor_mul(c1[:], a1[1][:], c1_p[:])
                nc.vector.tensor_add(c1[:], c1[:], tig1[:])
            c1_p = c1
            tch1 = spool.tile([BL, H], DT.float32, tag="tch", name=f"tch1_{t}")
            nc.scalar.activation(tch1[:], c1[:], AF.Tanh)
            h1 = spool.tile([BL, H], DT.float32, tag="h", name=f"h1_{t}")
            nc.vector.tensor_mul(h1[:], a1[3][:], tch1[:])
            h1T = hpool.tile([128, KC * BL], MM_DT, tag="h1T", name=f"h1T_{t}")
            for k in range(KC):
                trt = trp.tile([128, BL], DT.float32, tag="tr", name=f"tr1_{t}_{k}")
                nc.tensor.transpose(trt[:], h1[:, k * 128:(k + 1) * 128], ident[:])
                if k % 2 == 0:
                    nc.vector.tensor_copy(h1T[:, k * BL:(k + 1) * BL], trt[:])
                else:
                    nc.scalar.copy(h1T[:, k * BL:(k + 1) * BL], trt[:])

            # --- fc matmuls now; nonlinear tail deferred to next iter ---
            fcps = fcp.tile([BL, V], DT.float32, tag="fc", name=f"fc_{t}")
            for k in range(KC):
                nc.tensor.matmul(fcps[:], (h1T[:, k * BL:(k + 1) * BL]),
                                 (w_fc[:, k * V:(k + 1) * V]),
                                 start=(k == 0), stop=(k == KC - 1))
            nc.vector.tensor_add(fcps[:], fcps[:], bfb[:])
            fc_pend = (fcps, t)

            h0T_p, h1T_p = h0T, h1T

        emit_fc(*fc_pend)

        # ---------------- probs phase: exp/softmax from lgbuf ----------------
        PC = 16
        for c0 in range(0, nsteps, PC):
            cw = min(PC, nsteps - c0)
            exc = spool.tile([BL, PC * V], DT.float32, tag="exc", name=f"exc_{c0}")
            nc.scalar.activation(exc[:, :cw * V], lgbuf[:, c0 * V:(c0 + cw) * V], AF.Exp)
            sec = spool.tile([BL, PC], DT.float32, tag="sec", name=f"sec_{c0}")
            nc.vector.reduce_sum(sec[:, :cw],
                                 exc[:, :cw * V].rearrange("p (t v) -> p t v", v=V),
                                 axis=mybir.AxisListType.X)
            rcc = spool.tile([BL, PC], DT.float32, tag="rcc", name=f"rcc_{c0}")
            nc.vector.reciprocal(rcc[:, :cw], sec[:, :cw])
            prc = spool.tile([BL, PC * V], DT.float32, tag="prc", name=f"prc_{c0}")
            for j in range(cw):
                nc.vector.tensor_scalar(prc[:, j * V:(j + 1) * V],
                                        exc[:, j * V:(j + 1) * V],
                                        rcc[:, j:j + 1], None, op0=ALU.mult)
            nc.sync.dma_start(o_probs[:, c0:c0 + cw, :], prc[:, :cw * V])

        # ---------------- epilogue ----------------
        slsum = spool.tile([BL, 1], DT.float32, tag="slsum")
        nc.vector.reduce_sum(slsum[:], nllb[:, :nsteps], axis=mybir.AxisListType.X,
                             negate=True)
        sl = spool.tile([BL, 1], DT.float32, tag="sl")
        nc.vector.tensor_mul(sl[:], slsum[:], invlen[:])
        nc.sync.dma_start(o_sloss[:], sl[:])

    return nc




def build_graph_v2(t_steps=T):
    """bf16 col-tiled variant: L0 matmuls on PE col-groups 0-1 (psum rows
    0:64), L1 matmuls on col-groups 2-3 (psum rows 64:128), interleaved in
    the PE queue so the two streams overlap in the array. Software pipeline:
    iteration p consumes the gate psums written at p-1 (g0 of step p, g1 of
    step p-1), builds h0T(p)/h1T(p-1) via 16-bit DMA transposes, then issues
    the next matmul wave. Matmul operands bf16; PSUM/cell state f32."""
    ts_ = t_steps
    nsteps = ts_ - 1
    BF = DT.bfloat16

    nc = bacc.Bacc(None, target_bir_lowering=False)

    p_x = nc.declare_dram_parameter("x_i", [BL, ts_], DT.int32, isOutput=False)
    p_xrow = nc.declare_dram_parameter("x_row", [1, nsteps * BL], DT.int32, isOutput=False)
    p_embT = nc.declare_dram_parameter("embT", [E, V], DT.float32, isOutput=False)
    p_emb = nc.declare_dram_parameter("emb_tab", [V, E], DT.float32, isOutput=False)
    p_wih0T = nc.declare_dram_parameter("wih0T", [E, G], DT.float32, isOutput=False)
    p_wh0T = nc.declare_dram_parameter("wh0T", [H, G], DT.float32, isOutput=False)
    p_wi1T = nc.declare_dram_parameter("wi1T", [H, G], DT.float32, isOutput=False)
    p_wh1T = nc.declare_dram_parameter("wh1T", [H, G], DT.float32, isOutput=False)
    p_wfcT = nc.declare_dram_parameter("wfcT", [H, V], DT.float32, isOutput=False)
    p_bi0 = nc.declare_dram_parameter("b_i0", [1, G], DT.float32, isOutput=False)
    p_bh0 = nc.declare_dram_parameter("b_h0", [1, G], DT.float32, isOutput=False)
    p_bi1 = nc.declare_dram_parameter("b_i1", [1, G], DT.float32, isOutput=False)
    p_bh1 = nc.declare_dram_parameter("b_h1", [1, G], DT.float32, isOutput=False)
    p_bfc = nc.declare_dram_parameter("b_fc", [1, V], DT.float32, isOutput=False)

    o_probs = nc.declare_dram_parameter("probs", [BL, nsteps, V], DT.float32, isOutput=True)
    o_emb = nc.declare_dram_parameter("emb", [BL, ts_, E], DT.float32, isOutput=True)
    o_sloss = nc.declare_dram_parameter("sloss", [BL, 1], DT.float32, isOutput=True)

    TPA = dict(tile_position=(0, 0))
    TPB = dict(tile_position=(0, 64))

    with ExitStack() as ctx:
        tc = ctx.enter_context(tile.TileContext(nc))
        wpool = ctx.enter_context(tc.tile_pool(name="wts", bufs=1))
        epool = ctx.enter_context(tc.tile_pool(name="embg", bufs=2))

        x_int = wpool.tile([BL, ts_], DT.int32, tag="x_int")
        nc.sync.dma_start(x_int[:], p_x[:])
        xf = wpool.tile([BL, ts_], DT.float32, tag="xf")
        nc.vector.tensor_copy(xf[:], x_int[:])

        iota_i = wpool.tile([BL, V], DT.int32, tag="iota_i")
        nc.gpsimd.iota(iota_i[:], pattern=[[1, V]], base=0, channel_multiplier=0)
        iota36 = wpool.tile([BL, V], DT.float32, tag="iota36")
        nc.vector.tensor_copy(iota36[:], iota_i[:])

        ones_f = wpool.tile([1, BL], DT.float32, tag="ones_f")
        nc.gpsimd.memset(ones_f[:], 1.0)
        ones_b = wpool.tile([1, BL], MM_DT, tag="ones_b")
        nc.vector.tensor_copy(ones_b[:], ones_f[:])
        ones_v = wpool.tile([1, V], MM_DT, tag="ones_v")
        nc.vector.tensor_copy(ones_v[:], ones_f[:, :V])

        w_hh0 = wpool.tile([128, KC * G], BF, tag="w_hh0")
        w_ih1 = wpool.tile([128, KC * G], BF, tag="w_ih1")
        w_hh1 = wpool.tile([128, KC * G], BF, tag="w_hh1")
        for k in range(KC):
            nc.gpsimd.dma_start(w_hh0[:, k * G:(k + 1) * G], p_wh0T[k * 128:(k + 1) * 128, :])
            nc.gpsimd.dma_start(w_ih1[:, k * G:(k + 1) * G], p_wi1T[k * 128:(k + 1) * 128, :])
            nc.gpsimd.dma_start(w_hh1[:, k * G:(k + 1) * G], p_wh1T[k * 128:(k + 1) * 128, :])
        w_fc = wpool.tile([128, KC * V], BF, tag="w_fc")
        for k in range(KC):
            nc.gpsimd.dma_start(w_fc[:, k * V:(k + 1) * V], p_wfcT[k * 128:(k + 1) * 128, :])

        bfc_sb = wpool.tile([1, V], MM_DT, tag="bfc_sb")
        nc.gpsimd.dma_start(bfc_sb[:], p_bfc[:])

        bb = wpool.tile([128, G], BF, tag="bb")           # L1 bias at rows 64:128
        bfb = wpool.tile([BL, V], DT.float32, tag="bfb")  # fc bias at rows 0:64
        lgbuf = wpool.tile([BL, max(nsteps, 2) * V], BF, tag="lgbuf")
        u_sb = wpool.tile([V, G], BF, tag="u_sb")
        ohT = wpool.tile([V, nsteps * BL], BF, tag="ohT")
        npad = wpool.tile([BL, ts_], DT.float32, tag="npad")
        invlen = wpool.tile([BL, 1], DT.float32, tag="invlen")
        nllb = wpool.tile([BL, max(nsteps, 2)], DT.float32, tag="nllb")
        h1T_zero = wpool.tile([128, KC * BL], BF, tag="h1T_zero")
        nc.gpsimd.memset(h1T_zero[:], 0.0)

        with tc.tile_pool(name="prep", bufs=2) as ppool, \
             tc.tile_pool(name="ppsum", bufs=2, space="PSUM") as ppsum:
            iotc_i = ppool.tile([BL, 1], DT.int32, tag="iotc_i", bufs=1)
            nc.gpsimd.iota(iotc_i[:], pattern=[[1, 1]], base=0, channel_multiplier=1)
            iotc = ppool.tile([BL, 1], DT.float32, tag="iotc", bufs=1)
            nc.vector.tensor_copy(iotc[:], iotc_i[:])

            embT_sb = ppool.tile([E, V], MM_DT, tag="embT_sb", bufs=1)
            nc.gpsimd.dma_start(embT_sb[:], p_embT[:])
            wih0_sb = ppool.tile([E, G], MM_DT, tag="wih0_sb", bufs=1)
            nc.gpsimd.dma_start(wih0_sb[:], p_wih0T[:])

            b0s = ppool.tile([1, G], MM_DT, tag="b0s", bufs=1)
            nc.gpsimd.dma_start(b0s[:], p_bi0[:])
            bt = ppool.tile([1, G], DT.float32, tag="btmp", bufs=1)
            nc.sync.dma_start(bt[:], p_bh0[:])
            nc.vector.tensor_add(b0s[:], b0s[:], bt[:])
            b1s = ppool.tile([1, G], MM_DT, tag="b1s", bufs=1)
            nc.gpsimd.dma_start(b1s[:], p_bi1[:])
            bt2 = ppool.tile([1, G], DT.float32, tag="btmp", bufs=1)
            nc.sync.dma_start(bt2[:], p_bh1[:])
            nc.vector.tensor_add(b1s[:], b1s[:], bt2[:])

            for n in range(NB):
                ps = ppsum.tile([V, 512], DT.float32, tag="pp", name=f"ups_{n}")
                nc.tensor.matmul(ps[:], embT_sb[:], wih0_sb[:, n * 512:(n + 1) * 512],
                                 start=True, stop=False)
                nc.tensor.matmul(ps[:], ones_v[:], b0s[:, n * 512:(n + 1) * 512],
                                 start=False, stop=True)
                nc.scalar.copy(u_sb[:, n * 512:(n + 1) * 512], ps[:])

            psb = ppsum.tile([BL, V], DT.float32, tag="ppb", bufs=2)
            nc.tensor.matmul(psb[:], ones_b[:], bfc_sb[:], start=True, stop=True)
            nc.scalar.copy(bfb[:], psb[:])

            bbt = ppool.tile([BL, G], BF, tag="bbt", bufs=1)
            for n in range(NB):
                ps = ppsum.tile([BL, 512], DT.float32, tag="ppb", name=f"bb_{n}")
                nc.tensor.matmul(ps[:], ones_b[:], b1s[:, n * 512:(n + 1) * 512],
                                 start=True, stop=True)
                nc.scalar.copy(bbt[:, n * 512:(n + 1) * 512], ps[:])
            nc.sync.dma_start(bb[64:128, :], bbt[:])

            ncols = nsteps * BL
            pos = 0
            ci = 0
            while pos < ncols:
                w = min(512, ncols - pos)
                xr_i = ppool.tile([1, 512], DT.int32, tag="xr_i", bufs=4, name=f"xri_{ci}")
                nc.sync.dma_start(xr_i[:, :w], p_xrow[:, pos:pos + w])
                xr_f = ppool.tile([1, 512], MM_DT, tag="xr_f", bufs=4, name=f"xrf_{ci}")
                nc.vector.tensor_copy(xr_f[:, :w], xr_i[:, :w])
                ps = ppsum.tile([V, 512], DT.float32, tag="pp", name=f"ohps_{ci}")
                nc.tensor.matmul(ps[:, :w], ones_v[:], xr_f[:, :w], start=True, stop=True)
                nc.vector.tensor_scalar(ohT[:, pos:pos + w], ps[:, :w], iotc[:, 0:1], None,
                                        op0=ALU.is_equal)
                pos += w
                ci += 1

            nc.vector.tensor_scalar(npad[:], xf[:], 0.0, None, op0=ALU.not_equal)
            lenr = ppool.tile([BL, 1], DT.float32, tag="lenr", bufs=1)
            nc.vector.reduce_sum(lenr[:], npad[:], axis=mybir.AxisListType.X)
            nc.vector.reciprocal(invlen[:], lenr[:])

            EC = 8
            for c0 in range(0, ts_, EC):
                cw = min(EC, ts_ - c0)
                ebuf = epool.tile([BL, EC * E], DT.float32, tag="ebuf", name=f"ebuf_{c0}")
                for j in range(cw):
                    tcol = c0 + j
                    nc.gpsimd.indirect_dma_start(
                        out=ebuf[:, j * E:(j + 1) * E],
                        out_offset=None,
                        in_=p_emb[:],
                        in_offset=IndirectOffsetOnAxis(ap=x_int[:, tcol:tcol + 1], axis=0),
                    )
                nc.sync.dma_start(o_emb[:, c0:c0 + cw, :], ebuf[:, :cw * E])

        # ---------------- scan pools ----------------
        spool = ctx.enter_context(tc.tile_pool(name="work", bufs=2))
        apool = ctx.enter_context(tc.tile_pool(name="acts", bufs=6))
        hpool = ctx.enter_context(tc.tile_pool(name="hT", bufs=3))
        cpool = ctx.enter_context(tc.tile_pool(name="cc", bufs=2))
        g0p = ctx.enter_context(tc.tile_pool(name="g0psum", bufs=3, space="PSUM"))
        g1p = ctx.enter_context(tc.tile_pool(name="g1psum", bufs=3, space="PSUM"))
        fcp = ctx.enter_context(tc.tile_pool(name="fcpsum", bufs=2, space="PSUM"))

        GFUNC = [AF.Sigmoid, AF.Sigmoid, AF.Tanh, AF.Sigmoid]

        def fc_tail(fcps, t_):
            lg = lgbuf[:, t_ * V:(t_ + 1) * V]
            nc.vector.tensor_scalar_max(lg, fcps[:], 0.0)
            oht = spool.tile([BL, V], DT.float32, tag="oht", name=f"oht_{t_}")
            nc.vector.tensor_scalar(oht[:], iota36[:], xf[:, t_ + 1:t_ + 2], None,
                                    op0=ALU.is_equal)
            pick = spool.tile([BL, V], DT.float32, tag="pick", name=f"pick_{t_}")
            nc.vector.tensor_mul(pick[:], lg, oht[:])
            nv = spool.tile([BL, 1], DT.float32, tag="nv", name=f"nv_{t_}")
            nc.vector.reduce_sum(nv[:], pick[:], axis=mybir.AxisListType.X)
            nc.vector.tensor_mul(nllb[:, t_:t_ + 1], nv[:], npad[:, t_ + 1:t_ + 2])

        # prologue: g0(0) = oh(0) @ U
        g0_prev = []
        for n in range(NB):
            pn = g0p.tile([BL, 512], DT.float32, tag="g0", name=f"g0m1_{n}")
            nc.tensor.matmul(pn[:], ohT[:, 0:BL], u_sb[:, n * 512:(n + 1) * 512],
                             start=True, stop=True, **TPA)
            g0_prev.append(pn)

        g1_prev = None
        c0_prev = None
        c1_prev = None
        h1T_fc = None

        for p in range(nsteps):
            last = (p == nsteps - 1)
            # --- L0 elementwise from g0_prev = g0(p): -> h0(p), h0T(p) ---
            a0 = []
            for n in range(NB):
                an = apool.tile([BL, 512], DT.float32, tag="a0", name=f"a0_{p}_{n}")
                nc.scalar.activation(an[:], g0_prev[n][:], GFUNC[n])
                a0.append(an)
            tig0 = spool.tile([BL, H], DT.float32, tag="tig0", name=f"tig0_{p}")
            nc.vector.tensor_mul(tig0[:], a0[0][:], a0[2][:])
            cc0 = cpool.tile([BL, H], DT.float32, tag="c0", name=f"c0_{p}")
            if c0_prev is None:
                nc.vector.tensor_copy(cc0[:], tig0[:])
            else:
                nc.vector.tensor_mul(cc0[:], a0[1][:], c0_prev[:])
                nc.vector.tensor_add(cc0[:], cc0[:], tig0[:])
            c0_prev = cc0
            tch0 = spool.tile([BL, H], DT.float32, tag="tch0", name=f"tch0_{p}")
            nc.scalar.activation(tch0[:], cc0[:], AF.Tanh)
            h0 = spool.tile([BL, H], BF, tag="h0", name=f"h0_{p}")
            nc.vector.tensor_mul(h0[:], a0[3][:], tch0[:])
            h0T = hpool.tile([128, KC * BL], BF, tag="h0T", name=f"h0T_{p}")
            for k in range(KC):
                nc.sync.dma_start(h0T[:, k * BL:(k + 1) * BL],
                                  h0[:, k * 128:(k + 1) * 128], transpose=True)

            # --- L1 elementwise from g1_prev = g1(p-1): -> h1(p-1), h1T(p-1) ---
            if g1_prev is not None:
                a1 = []
                for n in range(NB):
                    an = apool.tile([128, 512], DT.float32, tag="a1", name=f"a1_{p}_{n}")
                    nc.scalar.activation(an[64:128, :], g1_prev[n][64:128, :], GFUNC[n])
                    a1.append(an)
                tig1 = spool.tile([128, H], DT.float32, tag="tig1", name=f"tig1_{p}")
                nc.vector.tensor_mul(tig1[64:128, :], a1[0][64:128, :], a1[2][64:128, :])
                cc1 = cpool.tile([128, H], DT.float32, tag="c1", name=f"c1_{p}")
                if c1_prev is None:
                    nc.vector.tensor_copy(cc1[64:128, :], tig1[64:128, :])
                else:
                    nc.vector.tensor_mul(cc1[64:128, :], a1[1][64:128, :], c1_prev[64:128, :])
                    nc.vector.tensor_add(cc1[64:128, :], cc1[64:128, :], tig1[64:128, :])
                c1_prev = cc1
                tch1 = spool.tile([128, H], DT.float32, tag="tch1", name=f"tch1_{p}")
                nc.scalar.activation(tch1[64:128, :], cc1[64:128, :], AF.Tanh)
                h1 = spool.tile([128, H], BF, tag="h1", name=f"h1_{p}")
                nc.vector.tensor_mul(h1[64:128, :], a1[3][64:128, :], tch1[64:128, :])
                h1T = hpool.tile([128, KC * BL], BF, tag="h1T", name=f"h1T_{p}")
                for k in range(KC):
                    nc.scalar.dma_start(h1T[:, k * BL:(k + 1) * BL],
                                        h1[64:128, k * 128:(k + 1) * 128], transpose=True)
            else:
                h1T = h1T_zero

            # --- matmul wave: A: g0(p+1) + fc(p-1); B: g1(p) ---
            g0_cur = [g0p.tile([BL, 512], DT.float32, tag="g0", name=f"g0_{p}_{n}")
                      for n in range(NB)] if not last else None
            g1_cur = [g1p.tile([128, 512], DT.float32, tag="g1", name=f"g1_{p}_{n}")
                      for n in range(NB)]
            amms = []
            if not last:
                for n in range(NB):
                    amms.append((g0_cur[n][:], ohT[:, (p + 1) * BL:(p + 2) * BL],
                                 u_sb[:, n * 512:(n + 1) * 512], True, False))
                    for k in range(KC):
                        amms.append((g0_cur[n][:], h0T[:, k * BL:(k + 1) * BL],
                                     w_hh0[:, k * G + n * 512:k * G + (n + 1) * 512],
                                     False, k == KC - 1))
            if p >= 1:
                fcps = fcp.tile([BL, V], DT.float32, tag="fc", name=f"fc_{p-1}")
                for k in range(KC):
                    amms.append((fcps[:], h1T[:, k * BL:(k + 1) * BL],
                                 w_fc[:, k * V:(k + 1) * V], k == 0, k == KC - 1))
            else:
                fcps = None
            bmms = []
            for n in range(NB):
                first = True
                if p >= 1:
                    for k in range(KC):
                        bmms.append((g1_cur[n][64:128, :], h1T[:, k * BL:(k + 1) * BL],
                                     w_hh1[:, k * G + n * 512:k * G + (n + 1) * 512],
                                     k == 0, False))
                    first = False
                for k in range(KC):
                    bmms.append((g1_cur[n][64:128, :], h0T[:, k * BL:(k + 1) * BL],
                                 w_ih1[:, k * G + n * 512:k * G + (n + 1) * 512],
                                 first and k == 0, k == KC - 1))
            for i in range(max(len(amms), len(bmms))):
                if i < len(bmms):
                    o, l, r, st, sp = bmms[i]
                    nc.tensor.matmul(o, l, r, start=st, stop=sp, **TPB)
                if i < len(amms):
                    o, l, r, st, sp = amms[i]
                    nc.tensor.matmul(o, l, r, start=st, stop=sp, **TPA)
            for n in range(NB):
                nc.vector.tensor_add(g1_cur[n][64:128, :], g1_cur[n][64:128, :],
                                     bb[64:128, n * 512:(n + 1) * 512])
            if fcps is not None:
                nc.vector.tensor_add(fcps[:], fcps[:], bfb[:])
                fc_tail(fcps, p - 1)

            g0_prev = g0_cur
            g1_prev = g1_cur

        # tail: g1(nsteps-1) -> h1(last), fc(last)
        a1 = []
        for n in range(NB):
            an = apool.tile([128, 512], DT.float32, tag="a1", name=f"a1_tail_{n}")
            nc.scalar.activation(an[64:128, :], g1_prev[n][64:128, :], GFUNC[n])
            a1.append(an)
        tig1 = spool.tile([128, H], DT.float32, tag="tig1", name="tig1_tail")
        nc.vector.tensor_mul(tig1[64:128, :], a1[0][64:128, :], a1[2][64:128, :])
        cc1 = cpool.tile([128, H], DT.float32, tag="c1", name="c1_tail")
        nc.vector.tensor_mul(cc1[64:128, :], a1[1][64:128, :], c1_prev[64:128, :])
        nc.vector.tensor_add(cc1[64:128, :], cc1[64:128, :], tig1[64:128, :])
        tch1 = spool.tile([128, H], DT.float32, tag="tch1", name="tch1_tail")
        nc.scalar.activation(tch1[64:128, :], cc1[64:128, :], AF.Tanh)
        h1 = spool.tile([128, H], BF, tag="h1", name="h1_tail")
        nc.vector.tensor_mul(h1[64:128, :], a1[3][64:128, :], tch1[64:128, :])
        h1T = hpool.tile([128, KC * BL], BF, tag="h1T", name="h1T_tail")
        for k in range(KC):
            nc.scalar.dma_start(h1T[:, k * BL:(k + 1) * BL],
                                h1[64:128, k * 128:(k + 1) * 128], transpose=True)
        fcps = fcp.tile([BL, V], DT.float32, tag="fc", name="fc_tail")
        for k in range(KC):
            nc.tensor.matmul(fcps[:], h1T[:, k * BL:(k + 1) * BL],
                             w_fc[:, k * V:(k + 1) * V],
                             start=(k == 0), stop=(k == KC - 1), **TPA)
        nc.vector.tensor_add(fcps[:], fcps[:], bfb[:])
        fc_tail(fcps, nsteps - 1)

        # probs phase
        PC = 16
        for c0 in range(0, nsteps, PC):
            cw = min(PC, nsteps - c0)
            exc = spool.tile([BL, PC * V], DT.float32, tag="exc", name=f"exc_{c0}")
            nc.scalar.activation(exc[:, :cw * V], lgbuf[:, c0 * V:(c0 + cw) * V], AF.Exp)
            sec = spool.tile([BL, PC], DT.float32, tag="sec", name=f"sec_{c0}")
            nc.vector.reduce_sum(sec[:, :cw],
                                 exc[:, :cw * V].rearrange("p (t v) -> p t v", v=V),
                                 axis=mybir.AxisListType.X)
            rcc = spool.tile([BL, PC], DT.float32, tag="rcc", name=f"rcc_{c0}")
            nc.vector.reciprocal(rcc[:, :cw], sec[:, :cw])
            prc = spool.tile([BL, PC * V], DT.float32, tag="prc", name=f"prc_{c0}")
            for j in range(cw):
                nc.vector.tensor_scalar(prc[:, j * V:(j + 1) * V],
                                        exc[:, j * V:(j + 1) * V],
                                        rcc[:, j:j + 1], None, op0=ALU.mult)
            nc.sync.dma_start(o_probs[:, c0:c0 + cw, :], prc[:, :cw * V])

        slsum = spool.tile([BL, 1], DT.float32, tag="slsum")
        nc.vector.reduce_sum(slsum[:], nllb[:, :nsteps], axis=mybir.AxisListType.X,
                             negate=True)
        sl = spool.tile([BL, 1], DT.float32, tag="sl")
        nc.vector.tensor_mul(sl[:], slsum[:], invlen[:])
        nc.sync.dma_start(o_sloss[:], sl[:])

    return nc


USE_V2 = os.environ.get("KV2", "0") == "1"


def build(t_steps=T):
    return build_graph_v2(t_steps) if USE_V2 else build_graph(t_steps)


def _prep_maps(x, emb_table, W_ih0, W_hh0, b_ih0, b_hh0,
               W_ih1, W_hh1, b_ih1, b_hh1, W_fc, b_fc, t_steps=T):
    """Host-side shard + layout prep (slicing / transposes / dtype casts only)."""
    f32 = np.float32
    x = np.asarray(x)
    xi = x.astype(np.int32)
    c = lambda a: np.ascontiguousarray(np.asarray(a), dtype=f32)
    shared = {
        "embT": c(np.asarray(emb_table).T),
        "emb_tab": c(emb_table),
        "wih0T": c(np.asarray(W_ih0).T),
        "wh0T": c(np.asarray(W_hh0).T),
        "wi1T": c(np.asarray(W_ih1).T),
        "wh1T": c(np.asarray(W_hh1).T),
        "wfcT": c(np.asarray(W_fc).T),
        "b_i0": c(b_ih0).reshape(1, -1),
        "b_h0": c(b_hh0).reshape(1, -1),
        "b_i1": c(b_ih1).reshape(1, -1),
        "b_h1": c(b_hh1).reshape(1, -1),
        "b_fc": c(b_fc).reshape(1, -1),
    }
    in_maps = []
    for i in range(NCORES):
        xs = xi[i * BL:(i + 1) * BL, :t_steps]
        m = dict(shared)
        m["x_i"] = np.ascontiguousarray(xs)
        m["x_row"] = np.ascontiguousarray(xs[:, :t_steps - 1].T.reshape(1, -1))
        in_maps.append(m)
    return in_maps


_NC_CACHE = {}


def kernel(x, emb_table, W_ih0, W_hh0, b_ih0, b_hh0,
           W_ih1, W_hh1, b_ih1, b_hh1, W_fc, b_fc):
    if T not in _NC_CACHE:
        nc_ = build(T)
        nc_.finalize()
        _NC_CACHE[T] = nc_
    nc = _NC_CACHE[T]
    in_maps = _prep_maps(x, emb_table, W_ih0, W_hh0, b_ih0, b_hh0,
                         W_ih1, W_hh1, b_ih1, b_hh1, W_fc, b_fc)
    res = run_bass_kernel_spmd(nc, in_maps, core_ids=list(range(NCORES)))
    outs = res.results
    probs = np.concatenate([o["probs"].reshape(BL, T - 1, V) for o in outs], axis=0)
    emb = np.concatenate([o["emb"].reshape(BL, T, E) for o in outs], axis=0)
    sloss = np.concatenate([o["sloss"].reshape(BL) for o in outs], axis=0)
    mean = np.float32(sloss.mean())
    return probs, emb, sloss, np.asarray(mean)


# revision 44
# speedup vs baseline: 1.0592x; 1.0592x over previous
"""Trainium2 Bass kernel for a 2-layer autoregressive LSTM LM.

Model: B=512, T=128, V=36, E=128, H=512.
  emb = emb_table[x]                                   [B, T, E]
  2-layer LSTM scan over T-1 steps (PyTorch gate order i,f,g,o)
  logits_t = relu(h1_t @ W_fc.T + b_fc)                [B, V]
  probs_t  = softmax(logits_t)
  nll_t    = -logits_t[n, tgt] masked at tgt==ignore(0)
  sample_loss = sum_t nll_t / count(x != 0)
Returns (probs [B,T-1,V], emb [B,T,E], sample_loss [B], mean_loss scalar).

Sharding: data-parallel over batch, 64 rows per core on 8 NeuronCores.
No collectives needed; mean_loss is reduced at host gather time.

Device-side structure (per core, single NEFF):
  prep:  U = emb_table @ W_ih0.T + (b_ih0+b_hh0)   [36, 2048] in SBUF
         ohT[v, t*64+b] = (x[b,t] == v)            one-hot.T via K=1
              broadcast-matmul + is_equal, for all 127 steps
         emb output rows gathered by indirect DMA from emb_table
  scan:  per step t (fully unrolled, batch on PSUM partitions M=64):
         g0 = ohT_t.T @ U + h0.T.T @ W_hh0.T       (PSUM accum, K=36+4x128)
         g1 = ones.T @ (b_ih1+b_hh1) + h0 @ W_ih1.T + h1 @ W_hh1.T
         gate nonlinearities on ACT straight from PSUM, c/h updates on DVE,
         h -> h.T via PE transposes (identity matmul) for the next step's
         stationary operand, fc/softmax/nll inline.
  All matmuls run as float32r (full-rate fp32 PE mode).
"""

import os
import numpy as np
from contextlib import ExitStack

import concourse.bass as bass
import concourse.mybir as mybir
import concourse.tile as tile
from concourse import bacc
from concourse.bass import IndirectOffsetOnAxis
from concourse.bass_utils import run_bass_kernel_spmd
from concourse.masks import make_identity

AF = mybir.ActivationFunctionType
ALU = mybir.AluOpType
DT = mybir.dt

B, T, V, E, H = 512, 128, 36, 128, 512
NCORES = 8
BL = B // NCORES          # 64 batch rows per core
G = 4 * H                 # 2048 gate width
NB = G // 512             # 4 psum-bank-wide gate chunks
KC = H // 128             # 4 K-chunks of the hidden dim

MM_DT = DT.float32r       # PE compute dtype (full-rate fp32 mode)


def _r_unused(ap):
    """Bitcast an f32 AP to the PE fast-fp32 dtype."""
    return ap.bitcast(MM_DT)


def build_graph(t_steps=T):
    """Build the SPMD single-core graph. t_steps is the number of x columns
    used (T in production; smaller for simulator smoke tests)."""
    ts_ = t_steps
    nsteps = ts_ - 1

    nc = bacc.Bacc(None, target_bir_lowering=False)

    # ---- DRAM parameters (per-core shards / replicated weights) ----
    p_x = nc.declare_dram_parameter("x_i", [BL, ts_], DT.int32, isOutput=False)
    # x[:, :nsteps] flattened t-major: [1, nsteps*BL]
    p_xrow = nc.declare_dram_parameter("x_row", [1, nsteps * BL], DT.int32, isOutput=False)
    p_embT = nc.declare_dram_parameter("embT", [E, V], DT.float32, isOutput=False)
    p_emb = nc.declare_dram_parameter("emb_tab", [V, E], DT.float32, isOutput=False)
    p_wih0T = nc.declare_dram_parameter("wih0T", [E, G], DT.float32, isOutput=False)
    p_wh0T = nc.declare_dram_parameter("wh0T", [H, G], DT.float32, isOutput=False)
    p_wi1T = nc.declare_dram_parameter("wi1T", [H, G], DT.float32, isOutput=False)
    p_wh1T = nc.declare_dram_parameter("wh1T", [H, G], DT.float32, isOutput=False)
    p_wfcT = nc.declare_dram_parameter("wfcT", [H, V], DT.float32, isOutput=False)
    p_bi0 = nc.declare_dram_parameter("b_i0", [1, G], DT.float32, isOutput=False)
    p_bh0 = nc.declare_dram_parameter("b_h0", [1, G], DT.float32, isOutput=False)
    p_bi1 = nc.declare_dram_parameter("b_i1", [1, G], DT.float32, isOutput=False)
    p_bh1 = nc.declare_dram_parameter("b_h1", [1, G], DT.float32, isOutput=False)
    p_bfc = nc.declare_dram_parameter("b_fc", [1, V], DT.float32, isOutput=False)

    o_probs = nc.declare_dram_parameter("probs", [BL, nsteps, V], DT.float32, isOutput=True)
    o_emb = nc.declare_dram_parameter("emb", [BL, ts_, E], DT.float32, isOutput=True)
    o_sloss = nc.declare_dram_parameter("sloss", [BL, 1], DT.float32, isOutput=True)

    with ExitStack() as ctx:
        tc = ctx.enter_context(tile.TileContext(nc))
        # persistent tensors (live for the whole kernel)
        wpool = ctx.enter_context(tc.tile_pool(name="wts", bufs=1))
        epool = ctx.enter_context(tc.tile_pool(name="embg", bufs=2))

        # ---------------- persistent constants + weights ----------------
        x_int = wpool.tile([BL, ts_], DT.int32, tag="x_int")
        nc.sync.dma_start(x_int[:], p_x[:])
        xf = wpool.tile([BL, ts_], DT.float32, tag="xf")
        nc.vector.tensor_copy(xf[:], x_int[:])

        ident = wpool.tile([BL, BL], DT.float32, tag="ident")
        make_identity(nc, ident[:])

        iota_i = wpool.tile([BL, V], DT.int32, tag="iota_i")
        nc.gpsimd.iota(iota_i[:], pattern=[[1, V]], base=0, channel_multiplier=0)
        iota36 = wpool.tile([BL, V], DT.float32, tag="iota36")
        nc.vector.tensor_copy(iota36[:], iota_i[:])

        ones_f = wpool.tile([1, BL], DT.float32, tag="ones_f")
        nc.gpsimd.memset(ones_f[:], 1.0)
        ones_b = wpool.tile([1, BL], MM_DT, tag="ones_b")
        nc.vector.tensor_copy(ones_b[:], ones_f[:])
        ones_v = wpool.tile([1, V], MM_DT, tag="ones_v")
        nc.vector.tensor_copy(ones_v[:], ones_f[:, :V])

        # weights to SBUF; K-chunk k of W.T lives at cols [k*G, (k+1)*G)
        # (weight DMAs are emitted in the prep scope after the prep-critical
        # small loads so the U/ohT builds are not queued behind 13 MB)
        w_hh0 = wpool.tile([128, KC * G], MM_DT, tag="w_hh0")
        w_ih1 = wpool.tile([128, KC * G], MM_DT, tag="w_ih1")
        w_hh1 = wpool.tile([128, KC * G], MM_DT, tag="w_hh1")
        w_fc = wpool.tile([128, KC * V], MM_DT, tag="w_fc")

        bfc_sb = wpool.tile([1, V], MM_DT, tag="bfc_sb")
        nc.gpsimd.dma_start(bfc_sb[:], p_bfc[:])

        bb = wpool.tile([BL, G], DT.bfloat16, tag="bb")
        bfb = wpool.tile([BL, V], DT.float32, tag="bfb")
        lgbuf = wpool.tile([BL, max(nsteps, 2) * V], DT.bfloat16, tag="lgbuf")
        u_sb = wpool.tile([BL, G], MM_DT, tag="u_sb")
        ohT = wpool.tile([BL, nsteps * BL], MM_DT, tag="ohT")
        npad = wpool.tile([BL, ts_], DT.float32, tag="npad")
        invlen = wpool.tile([BL, 1], DT.float32, tag="invlen")
        nllb = wpool.tile([BL, max(nsteps, 2)], DT.float32, tag="nllb")

        # ---------------- prep scope (released before the scan) ----------------
        with tc.tile_pool(name="prep", bufs=2) as ppool, \
             tc.tile_pool(name="ppsum", bufs=2, space="PSUM") as ppsum:
            zrow = ppool.tile([32, G], DT.float32, tag="btmp", bufs=1)
            nc.gpsimd.memset(zrow[:], 0.0)
            nc.scalar.copy(u_sb[32:BL, :], zrow[:])
            iotc_i = ppool.tile([BL, 1], DT.int32, tag="iotc_i", bufs=1)
            nc.gpsimd.iota(iotc_i[:], pattern=[[1, 1]], base=0, channel_multiplier=1)
            iotc = ppool.tile([BL, 1], DT.float32, tag="iotc", bufs=1)
            nc.vector.tensor_copy(iotc[:], iotc_i[:])

            embT_sb = ppool.tile([E, V], MM_DT, tag="embT_sb", bufs=1)
            nc.gpsimd.dma_start(embT_sb[:], p_embT[:])
            wih0_sb = ppool.tile([E, G], MM_DT, tag="wih0_sb", bufs=1)
            nc.gpsimd.dma_start(wih0_sb[:], p_wih0T[:])

            # bias sums BEFORE the big weight DMAs on the gpsimd queue, so the
            # U/bias-broadcast builds are not stalled behind 13 MB of weights
            b0s = ppool.tile([1, G], MM_DT, tag="b0s", bufs=1)
            nc.gpsimd.dma_start(b0s[:], p_bi0[:])
            bt = ppool.tile([1, G], DT.float32, tag="btmp", bufs=1)
            nc.sync.dma_start(bt[:], p_bh0[:])
            nc.vector.tensor_add(b0s[:], b0s[:], bt[:])
            b1s = ppool.tile([1, G], MM_DT, tag="b1s", bufs=1)
            nc.gpsimd.dma_start(b1s[:], p_bi1[:])
            bt2 = ppool.tile([1, G], DT.float32, tag="btmp", bufs=1)
            nc.sync.dma_start(bt2[:], p_bh1[:])
            nc.vector.tensor_add(b1s[:], b1s[:], bt2[:])

            for k in range(KC):
                nc.gpsimd.dma_start(w_hh0[:, k * G:(k + 1) * G], p_wh0T[k * 128:(k + 1) * 128, :])
                nc.gpsimd.dma_start(w_ih1[:, k * G:(k + 1) * G], p_wi1T[k * 128:(k + 1) * 128, :])
                nc.gpsimd.dma_start(w_hh1[:, k * G:(k + 1) * G], p_wh1T[k * 128:(k + 1) * 128, :])
            for k in range(KC):
                nc.gpsimd.dma_start(w_fc[:, k * V:(k + 1) * V], p_wfcT[k * 128:(k + 1) * 128, :])

            # U = emb_table @ W_ih0.T + b0s : [V, G] (rows V..BL stay zero)
            for n in range(NB):
                ps = ppsum.tile([V, 512], DT.float32, tag="pp", name=f"ups_{n}")
                nc.tensor.matmul(ps[:], (embT_sb[:]), (wih0_sb[:, n * 512:(n + 1) * 512]),
                                 start=True, stop=False)
                nc.tensor.matmul(ps[:], (ones_v[:]), (b0s[:, n * 512:(n + 1) * 512]),
                                 start=False, stop=True)
                nc.scalar.copy(u_sb[0:V, n * 512:(n + 1) * 512], ps[:])

            # ohT[v, t*BL+b] = (x[b, t] == v) for the input steps
            ncols = nsteps * BL
            pos = 0
            ci = 0
            while pos < ncols:
                w = min(512, ncols - pos)
                xr_i = ppool.tile([1, 512], DT.int32, tag="xr_i", bufs=4, name=f"xri_{ci}")
                nc.sync.dma_start(xr_i[:, :w], p_xrow[:, pos:pos + w])
                xr_f = ppool.tile([1, 512], MM_DT, tag="xr_f", bufs=4, name=f"xrf_{ci}")
                nc.vector.tensor_copy(xr_f[:, :w], xr_i[:, :w])
                ps = ppsum.tile([BL, 512], DT.float32, tag="pp", name=f"ohps_{ci}")
                nc.tensor.matmul(ps[:, :w], (ones_b[:]), (xr_f[:, :w]),
                                 start=True, stop=True)
                nc.vector.tensor_scalar(ohT[:, pos:pos + w], ps[:, :w], iotc[:, 0:1], None,
                                        op0=ALU.is_equal)
                pos += w
                ci += 1

            # broadcast fc bias to all batch partitions
            psb = ppsum.tile([BL, V], DT.float32, tag="ppb", bufs=2)
            nc.tensor.matmul(psb[:], (ones_b[:]), (bfc_sb[:]), start=True, stop=True)
            nc.scalar.copy(bfb[:], psb[:])

            # broadcast L1 bias to all batch partitions (bf16 is plenty)
            for n in range(NB):
                ps = ppsum.tile([BL, 512], DT.float32, tag="ppb", name=f"bb_{n}")
                nc.tensor.matmul(ps[:], (ones_b[:]), (b1s[:, n * 512:(n + 1) * 512]),
                                 start=True, stop=True)
                nc.scalar.copy(bb[:, n * 512:(n + 1) * 512], ps[:])

            # pad mask + inverse length
            nc.vector.tensor_scalar(npad[:], xf[:], 0.0, None, op0=ALU.not_equal)
            lenr = ppool.tile([BL, 1], DT.float32, tag="lenr", bufs=1)
            nc.vector.reduce_sum(lenr[:], npad[:], axis=mybir.AxisListType.X)
            nc.vector.reciprocal(invlen[:], lenr[:])

            # emb output: gather rows of emb_table by token id, chunked DMA out
            EC = 4  # t's per chunk
            for c0 in range(0, ts_, EC):
                cw = min(EC, ts_ - c0)
                ebuf = epool.tile([BL, EC * E], DT.float32, tag="ebuf", name=f"ebuf_{c0}")
                for j in range(cw):
                    tcol = c0 + j
                    nc.gpsimd.indirect_dma_start(
                        out=ebuf[:, j * E:(j + 1) * E],
                        out_offset=None,
                        in_=p_emb[:],
                        in_offset=IndirectOffsetOnAxis(ap=x_int[:, tcol:tcol + 1], axis=0),
                    )
                nc.sync.dma_start(o_emb[:, c0:c0 + cw, :], ebuf[:, :cw * E])

        # ---------------- scan pools ----------------
        spool = ctx.enter_context(tc.tile_pool(name="work", bufs=2))
        apool = ctx.enter_context(tc.tile_pool(name="acts", bufs=6))
        hpool = ctx.enter_context(tc.tile_pool(name="hT", bufs=4))
        cpool = ctx.enter_context(tc.tile_pool(name="cc", bufs=2))
        gp = ctx.enter_context(tc.tile_pool(name="gpsum", bufs=3, space="PSUM"))
        trp = ctx.enter_context(tc.tile_pool(name="trpsum", bufs=4, space="PSUM"))
        fcp = ctx.enter_context(tc.tile_pool(name="fcpsum", bufs=1, space="PSUM"))

        GFUNC = [AF.Sigmoid, AF.Sigmoid, AF.Tanh, AF.Sigmoid]  # i, f, g, o

        h0T_p = None  # [128, KC*BL] transposed hidden, layer 0, prev step
        h1T_p = None
        c0_p = None
        c1_p = None
        fc_pend = None  # (psum, t) for the deferred fc tail of step t-1

        def emit_fc(fcps, t_):
            # relu on DVE straight into the bf16 logits buffer (no ACT table swap)
            lg = lgbuf[:, t_ * V:(t_ + 1) * V]
            nc.vector.tensor_scalar_max(lg, fcps[:], 0.0)
            # nll: logits[tgt] * (tgt != 0), accumulated for the end
            oht = spool.tile([BL, V], DT.float32, tag="oht", name=f"oht_{t_}")
            nc.vector.tensor_scalar(oht[:], iota36[:], xf[:, t_ + 1:t_ + 2], None,
                                    op0=ALU.is_equal)
            pick = spool.tile([BL, V], DT.float32, tag="pick", name=f"pick_{t_}")
            nc.vector.tensor_mul(pick[:], lg, oht[:])
            nv = spool.tile([BL, 1], DT.float32, tag="nv", name=f"nv_{t_}")
            nc.vector.reduce_sum(nv[:], pick[:], axis=mybir.AxisListType.X)
            nc.vector.tensor_mul(nllb[:, t_:t_ + 1], nv[:], npad[:, t_ + 1:t_ + 2])

        for t in range(nsteps):
            oh_t = ohT[:, t * BL:(t + 1) * BL]

            # --- L0 matmuls: g0 = oh_t.T @ U (+ h0T_p.T @ Whh0.T) ---
            g0 = [gp.tile([BL, 512], DT.float32, tag="g", name=f"g0_{t}_{n}") for n in range(NB)]
            for n in range(NB):
                nc.tensor.matmul(g0[n][:], (oh_t), (u_sb[:, n * 512:(n + 1) * 512]),
                                 start=True, stop=(h0T_p is None))
                if h0T_p is not None:
                    for k in range(KC):
                        nc.tensor.matmul(
                            g0[n][:], (h0T_p[:, k * BL:(k + 1) * BL]),
                            (w_hh0[:, k * G + n * 512:k * G + (n + 1) * 512]),
                            start=False, stop=(k == KC - 1))

            # --- L1 matmuls part 1: bias + Whh1 (uses h1T_p) ---
            g1 = [gp.tile([BL, 512], DT.float32, tag="g", name=f"g1_{t}_{n}") for n in range(NB)]
            if h1T_p is not None:
                for n in range(NB):
                    for k in range(KC):
                        nc.tensor.matmul(
                            g1[n][:], (h1T_p[:, k * BL:(k + 1) * BL]),
                            (w_hh1[:, k * G + n * 512:k * G + (n + 1) * 512]),
                            start=(k == 0), stop=False)

            # --- deferred fc tail of the previous step (keeps PE dense) ---
            if fc_pend is not None:
                emit_fc(*fc_pend)
                fc_pend = None

            # --- L0 elementwise: gates -> c0, h0, h0T ---
            a0 = []
            for n in range(NB):
                an = apool.tile([BL, 512], DT.float32, tag="a", name=f"a0_{t}_{n}")
                nc.scalar.activation(an[:], g0[n][:], GFUNC[n])
                a0.append(an)
            tig = spool.tile([BL, H], DT.float32, tag="tig", name=f"tig0_{t}")
            nc.vector.tensor_mul(tig[:], a0[0][:], a0[2][:])      # i*tanh(g)
            c0 = cpool.tile([BL, H], DT.float32, tag="c0", name=f"c0_{t}")
            if c0_p is None:
                nc.vector.tensor_copy(c0[:], tig[:])
            else:
                nc.vector.tensor_mul(c0[:], a0[1][:], c0_p[:])    # f*c
                nc.vector.tensor_add(c0[:], c0[:], tig[:])
            c0_p = c0
            tch = spool.tile([BL, H], DT.float32, tag="tch", name=f"tch0_{t}")
            nc.scalar.activation(tch[:], c0[:], AF.Tanh)
            h0 = spool.tile([BL, H], DT.float32, tag="h", name=f"h0_{t}")
            nc.vector.tensor_mul(h0[:], a0[3][:], tch[:])         # o*tanh(c)
            h0T = hpool.tile([128, KC * BL], MM_DT, tag="h0T", name=f"h0T_{t}")
            for k in range(KC):
                trt = trp.tile([128, BL], DT.float32, tag="tr", name=f"tr0_{t}_{k}")
                nc.tensor.transpose(trt[:], h0[:, k * 128:(k + 1) * 128], ident[:])
                if k % 2 == 0:
                    nc.vector.tensor_copy(h0T[:, k * BL:(k + 1) * BL], trt[:])
                else:
                    nc.scalar.copy(h0T[:, k * BL:(k + 1) * BL], trt[:])

            # --- L1 matmuls part 2: W_ih1 with fresh h0T (closes accum) ---
            for n in range(NB):
                for k in range(KC):
                    nc.tensor.matmul(
                        g1[n][:], (h0T[:, k * BL:(k + 1) * BL]),
                        (w_ih1[:, k * G + n * 512:k * G + (n + 1) * 512]),
                        start=(h1T_p is None and k == 0), stop=(k == KC - 1))
                # bias add in PSUM (DVE), instead of a K=1 matmul on PE
                nc.vector.tensor_add(g1[n][:], g1[n][:], bb[:, n * 512:(n + 1) * 512])

            # --- L1 elementwise ---
            a1 = []
            for n in range(NB):
                an = apool.tile([BL, 512], DT.float32, tag="a", name=f"a1_{t}_{n}")
                nc.scalar.activation(an[:], g1[n][:], GFUNC[n])
                a1.append(an)
            tig1 = spool.tile([BL, H], DT.float32, tag="tig", name=f"tig1_{t}")
            nc.vector.tensor_mul(tig1[:], a1[0][:], a1[2][:])
            c1 = cpool.tile([BL, H], DT.float32, tag="c1", name=f"c1_{t}")
            if c1_p is None:
                nc.vector.tensor_copy(c1[:], tig1[:])
            else:
                nc.vector.tensor_mul(c1[:], a1[1][:], c1_p[:])
                nc.vector.tensor_add(c1[:], c1[:], tig1[:])
            c1_p = c1
            tch1 = spool.tile([BL, H], DT.float32, tag="tch", name=f"tch1_{t}")
            nc.scalar.activation(tch1[:], c1[:], AF.Tanh)
            h1 = spool.tile([BL, H], DT.float32, tag="h", name=f"h1_{t}")
            nc.vector.tensor_mul(h1[:], a1[3][:], tch1[:])
            h1T = hpool.tile([128, KC * BL], MM_DT, tag="h1T", name=f"h1T_{t}")
            for k in range(KC):
                trt = trp.tile([128, BL], DT.float32, tag="tr", name=f"tr1_{t}_{k}")
                nc.tensor.transpose(trt[:], h1[:, k * 128:(k + 1) * 128], ident[:])
                if k % 2 == 0:
                    nc.vector.tensor_copy(h1T[:, k * BL:(k + 1) * BL], trt[:])
                else:
                    nc.scalar.copy(h1T[:, k * BL:(k + 1) * BL], trt[:])

            # --- fc matmuls now; nonlinear tail deferred to next iter ---
            fcps = fcp.tile([BL, V], DT.float32, tag="fc", name=f"fc_{t}")
            for k in range(KC):
                nc.tensor.matmul(fcps[:], (h1T[:, k * BL:(k + 1) * BL]),
                                 (w_fc[:, k * V:(k + 1) * V]),
                                 start=(k == 0), stop=(k == KC - 1))
            nc.vector.tensor_add(fcps[:], fcps[:], bfb[:])
            fc_pend = (fcps, t)

            h0T_p, h1T_p = h0T, h1T

        emit_fc(*fc_pend)

        # ---------------- probs phase: exp/softmax from lgbuf ----------------
        PC = 16
        for c0 in range(0, nsteps, PC):
            cw = min(PC, nsteps - c0)
            exc = spool.tile([BL, PC * V], DT.float32, tag="exc", name=f"exc_{c0}")
            nc.scalar.activation(exc[:, :cw * V], lgbuf[:, c0 * V:(c0 + cw) * V], AF.Exp)
            sec = spool.tile([BL, PC], DT.float32, tag="sec", name=f"sec_{c0}")
            nc.vector.reduce_sum(sec[:, :cw],
                                 exc[:, :cw * V].rearrange("p (t v) -> p t v", v=V),
                                 axis=mybir.AxisListType.X)
            rcc = spool.tile([BL, PC], DT.float32, tag="rcc", name=f"rcc_{c0}")
            nc.vector.reciprocal(rcc[:, :cw], sec[:, :cw])
            prc = spool.tile([BL, PC * V], DT.float32, tag="prc", name=f"prc_{c0}")
            for j in range(cw):
                nc.vector.tensor_scalar(prc[:, j * V:(j + 1) * V],
                                        exc[:, j * V:(j + 1) * V],
                                        rcc[:, j:j + 1], None, op0=ALU.mult)
            nc.sync.dma_start(o_probs[:, c0:c0 + cw, :], prc[:, :cw * V])

        # ---------------- epilogue ----------------
        slsum = spool.tile([BL, 1], DT.float32, tag="slsum")
        nc.vector.reduce_sum(slsum[:], nllb[:, :nsteps], axis=mybir.AxisListType.X,
                             negate=True)
        sl = spool.tile([BL, 1], DT.float32, tag="sl")
        nc.vector.tensor_mul(sl[:], slsum[:], invlen[:])
        nc.sync.dma_start(o_sloss[:], sl[:])

    return nc




def build_graph_v2(t_steps=T):
    """bf16 col-tiled variant: L0 matmuls on PE col-groups 0-1 (psum rows
    0:64), L1 matmuls on col-groups 2-3 (psum rows 64:128), interleaved in
    the PE queue so the two streams overlap in the array. Software pipeline:
    iteration p consumes the gate psums written at p-1 (g0 of step p, g1 of
    step p-1), builds h0T(p)/h1T(p-1) via 16-bit DMA transposes, then issues
    the next matmul wave. Matmul operands bf16; PSUM/cell state f32."""
    ts_ = t_steps
    nsteps = ts_ - 1
    BF = DT.bfloat16

    nc = bacc.Bacc(None, target_bir_lowering=False)

    p_x = nc.declare_dram_parameter("x_i", [BL, ts_], DT.int32, isOutput=False)
    p_xrow = nc.declare_dram_parameter("x_row", [1, nsteps * BL], DT.int32, isOutput=False)
    p_embT = nc.declare_dram_parameter("embT", [E, V], DT.float32, isOutput=False)
    p_emb = nc.declare_dram_parameter("emb_tab", [V, E], DT.float32, isOutput=False)
    p_wih0T = nc.declare_dram_parameter("wih0T", [E, G], DT.float32, isOutput=False)
    p_wh0T = nc.declare_dram_parameter("wh0T", [H, G], DT.float32, isOutput=False)
    p_wi1T = nc.declare_dram_parameter("wi1T", [H, G], DT.float32, isOutput=False)
    p_wh1T = nc.declare_dram_parameter("wh1T", [H, G], DT.float32, isOutput=False)
    p_wfcT = nc.declare_dram_parameter("wfcT", [H, V], DT.float32, isOutput=False)
    p_bi0 = nc.declare_dram_parameter("b_i0", [1, G], DT.float32, isOutput=False)
    p_bh0 = nc.declare_dram_parameter("b_h0", [1, G], DT.float32, isOutput=False)
    p_bi1 = nc.declare_dram_parameter("b_i1", [1, G], DT.float32, isOutput=False)
    p_bh1 = nc.declare_dram_parameter("b_h1", [1, G], DT.float32, isOutput=False)
    p_bfc = nc.declare_dram_parameter("b_fc", [1, V], DT.float32, isOutput=False)

    o_probs = nc.declare_dram_parameter("probs", [BL, nsteps, V], DT.float32, isOutput=True)
    o_emb = nc.declare_dram_parameter("emb", [BL, ts_, E], DT.float32, isOutput=True)
    o_sloss = nc.declare_dram_parameter("sloss", [BL, 1], DT.float32, isOutput=True)

    TPA = dict(tile_position=(0, 0))
    TPB = dict(tile_position=(0, 64))

    with ExitStack() as ctx:
        tc = ctx.enter_context(tile.TileContext(nc))
        wpool = ctx.enter_context(tc.tile_pool(name="wts", bufs=1))
        epool = ctx.enter_context(tc.tile_pool(name="embg", bufs=2))

        x_int = wpool.tile([BL, ts_], DT.int32, tag="x_int")
        nc.sync.dma_start(x_int[:], p_x[:])
        xf = wpool.tile([BL, ts_], DT.float32, tag="xf")
        nc.vector.tensor_copy(xf[:], x_int[:])

        iota_i = wpool.tile([BL, V], DT.int32, tag="iota_i")
        nc.gpsimd.iota(iota_i[:], pattern=[[1, V]], base=0, channel_multiplier=0)
        iota36 = wpool.tile([BL, V], DT.float32, tag="iota36")
        nc.vector.tensor_copy(iota36[:], iota_i[:])

        ones_f = wpool.tile([1, BL], DT.float32, tag="ones_f")
        nc.gpsimd.memset(ones_f[:], 1.0)
        ones_b = wpool.tile([1, BL], MM_DT, tag="ones_b")
        nc.vector.tensor_copy(ones_b[:], ones_f[:])
        ones_v = wpool.tile([1, V], MM_DT, tag="ones_v")
        nc.vector.tensor_copy(ones_v[:], ones_f[:, :V])

        w_hh0 = wpool.tile([128, KC * G], BF, tag="w_hh0")
        w_ih1 = wpool.tile([128, KC * G], BF, tag="w_ih1")
        w_hh1 = wpool.tile([128, KC * G], BF, tag="w_hh1")
        for k in range(KC):
            nc.gpsimd.dma_start(w_hh0[:, k * G:(k + 1) * G], p_wh0T[k * 128:(k + 1) * 128, :])
            nc.gpsimd.dma_start(w_ih1[:, k * G:(k + 1) * G], p_wi1T[k * 128:(k + 1) * 128, :])
            nc.gpsimd.dma_start(w_hh1[:, k * G:(k + 1) * G], p_wh1T[k * 128:(k + 1) * 128, :])
        w_fc = wpool.tile([128, KC * V], BF, tag="w_fc")
        for k in range(KC):
            nc.gpsimd.dma_start(w_fc[:, k * V:(k + 1) * V], p_wfcT[k * 128:(k + 1) * 128, :])

        bfc_sb = wpool.tile([1, V], MM_DT, tag="bfc_sb")
        nc.gpsimd.dma_start(bfc_sb[:], p_bfc[:])

        bb = wpool.tile([128, G], BF, tag="bb")           # L1 bias at rows 64:128
        bfb = wpool.tile([BL, V], DT.float32, tag="bfb")  # fc bias at rows 0:64
        lgbuf = wpool.tile([BL, max(nsteps, 2) * V], BF, tag="lgbuf")
        u_sb = wpool.tile([V, G], BF, tag="u_sb")
        ohT = wpool.tile([V, nsteps * BL], BF, tag="ohT")
        npad = wpool.tile([BL, ts_], DT.float32, tag="npad")
        invlen = wpool.tile([BL, 1], DT.float32, tag="invlen")
        nllb = wpool.tile([BL, max(nsteps, 2)], DT.float32, tag="nllb")
        h1T_zero = wpool.tile([128, KC * BL], BF, tag="h1T_zero")
        nc.gpsimd.memset(h1T_zero[:], 0.0)

        with tc.tile_pool(name="prep", bufs=2) as ppool, \
             tc.tile_pool(name="ppsum", bufs=2, space="PSUM") as ppsum:
            iotc_i = ppool.tile([BL, 1], DT.int32, tag="iotc_i", bufs=1)
            nc.gpsimd.iota(iotc_i[:], pattern=[[1, 1]], base=0, channel_multiplier=1)
            iotc = ppool.tile([BL, 1], DT.float32, tag="iotc", bufs=1)
            nc.vector.tensor_copy(iotc[:], iotc_i[:])

            embT_sb = ppool.tile([E, V], MM_DT, tag="embT_sb", bufs=1)
            nc.gpsimd.dma_start(embT_sb[:], p_embT[:])
            wih0_sb = ppool.tile([E, G], MM_DT, tag="wih0_sb", bufs=1)
            nc.gpsimd.dma_start(wih0_sb[:], p_wih0T[:])

            b0s = ppool.tile([1, G], MM_DT, tag="b0s", bufs=1)
            nc.gpsimd.dma_start(b0s[:], p_bi0[:])
            bt = ppool.tile([1, G], DT.float32, tag="btmp", bufs=1)
            nc.sync.dma_start(bt[:], p_bh0[:])
            nc.vector.tensor_add(b0s[:], b0s[:], bt[:])
            b1s = ppool.tile([1, G], MM_DT, tag="b1s", bufs=1)
            nc.gpsimd.dma_start(b1s[:], p_bi1[:])
            bt2 = ppool.tile([1, G], DT.float32, tag="btmp", bufs=1)
            nc.sync.dma_start(bt2[:], p_bh1[:])
            nc.vector.tensor_add(b1s[:], b1s[:], bt2[:])

            for n in range(NB):
                ps = ppsum.tile([V, 512], DT.float32, tag="pp", name=f"ups_{n}")
                nc.tensor.matmul(ps[:], embT_sb[:], wih0_sb[:, n * 512:(n + 1) * 512],
                                 start=True, stop=False)
                nc.tensor.matmul(ps[:], ones_v[:], b0s[:, n * 512:(n + 1) * 512],
                                 start=False, stop=True)
                nc.scalar.copy(u_sb[:, n * 512:(n + 1) * 512], ps[:])

            psb = ppsum.tile([BL, V], DT.float32, tag="ppb", bufs=2)
            nc.tensor.matmul(psb[:], ones_b[:], bfc_sb[:], start=True, stop=True)
            nc.scalar.copy(bfb[:], psb[:])

            bbt = ppool.tile([BL, G], BF, tag="bbt", bufs=1)
            for n in range(NB):
                ps = ppsum.tile([BL, 512], DT.float32, tag="ppb", name=f"bb_{n}")
                nc.tensor.matmul(ps[:], ones_b[:], b1s[:, n * 512:(n + 1) * 512],
                                 start=True, stop=True)
                nc.scalar.copy(bbt[:, n * 512:(n + 1) * 512], ps[:])
            nc.sync.dma_start(bb[64:128, :], bbt[:])

            ncols = nsteps * BL
            pos = 0
            ci = 0
            while pos < ncols:
                w = min(512, ncols - pos)
                xr_i = ppool.tile([1, 512], DT.int32, tag="xr_i", bufs=4, name=f"xri_{ci}")
                nc.sync.dma_start(xr_i[:, :w], p_xrow[:, pos:pos + w])
                xr_f = ppool.tile([1, 512], MM_DT, tag="xr_f", bufs=4, name=f"xrf_{ci}")
                nc.vector.tensor_copy(xr_f[:, :w], xr_i[:, :w])
                ps = ppsum.tile([V, 512], DT.float32, tag="pp", name=f"ohps_{ci}")
                nc.tensor.matmul(ps[:, :w], ones_v[:], xr_f[:, :w], start=True, stop=True)
                nc.vector.tensor_scalar(ohT[:, pos:pos + w], ps[:, :w], iotc[:, 0:1], None,
                                        op0=ALU.is_equal)
                pos += w
                ci += 1

            nc.vector.tensor_scalar(npad[:], xf[:], 0.0, None, op0=ALU.not_equal)
            lenr = ppool.tile([BL, 1], DT.float32, tag="lenr", bufs=1)
            nc.vector.reduce_sum(lenr[:], npad[:], axis=mybir.AxisListType.X)
            nc.vector.reciprocal(invlen[:], lenr[:])

            EC = 8
            for c0 in range(0, ts_, EC):
                cw = min(EC, ts_ - c0)
                ebuf = epool.tile([BL, EC * E], DT.float32, tag="ebuf", name=f"ebuf_{c0}")
                for j in range(cw):
                    tcol = c0 + j
                    nc.gpsimd.indirect_dma_start(
                        out=ebuf[:, j * E:(j + 1) * E],
                        out_offset=None,
                        in_=p_emb[:],
                        in_offset=IndirectOffsetOnAxis(ap=x_int[:, tcol:tcol + 1], axis=0),
                    )
                nc.sync.dma_start(o_emb[:, c0:c0 + cw, :], ebuf[:, :cw * E])

        # ---------------- scan pools ----------------
        spool = ctx.enter_context(tc.tile_pool(name="work", bufs=2))
        apool = ctx.enter_context(tc.tile_pool(name="acts", bufs=6))
        hpool = ctx.enter_context(tc.tile_pool(name="hT", bufs=3))
        cpool = ctx.enter_context(tc.tile_pool(name="cc", bufs=2))
        g0p = ctx.enter_context(tc.tile_pool(name="g0psum", bufs=3, space="PSUM"))
        g1p = ctx.enter_context(tc.tile_pool(name="g1psum", bufs=3, space="PSUM"))
        fcp = ctx.enter_context(tc.tile_pool(name="fcpsum", bufs=2, space="PSUM"))

        GFUNC = [AF.Sigmoid, AF.Sigmoid, AF.Tanh, AF.Sigmoid]

        def fc_tail(fcps, t_):
            lg = lgbuf[:, t_ * V:(t_ + 1) * V]
            nc.vector.tensor_scalar_max(lg, fcps[:], 0.0)
            oht = spool.tile([BL, V], DT.float32, tag="oht", name=f"oht_{t_}")
            nc.vector.tensor_scalar(oht[:], iota36[:], xf[:, t_ + 1:t_ + 2], None,
                                    op0=ALU.is_equal)
            pick = spool.tile([BL, V], DT.float32, tag="pick", name=f"pick_{t_}")
            nc.vector.tensor_mul(pick[:], lg, oht[:])
            nv = spool.tile([BL, 1], DT.float32, tag="nv", name=f"nv_{t_}")
            nc.vector.reduce_sum(nv[:], pick[:], axis=mybir.AxisListType.X)
            nc.vector.tensor_mul(nllb[:, t_:t_ + 1], nv[:], npad[:, t_ + 1:t_ + 2])

        # prologue: g0(0) = oh(0) @ U
        g0_prev = []
        for n in range(NB):
            pn = g0p.tile([BL, 512], DT.float32, tag="g0", name=f"g0m1_{n}")
            nc.tensor.matmul(pn[:], ohT[:, 0:BL], u_sb[:, n * 512:(n + 1) * 512],
                             start=True, stop=True, **TPA)
            g0_prev.append(pn)

        g1_prev = None
        c0_prev = None
        c1_prev = None
        h1T_fc = None

        for p in range(nsteps):
            last = (p == nsteps - 1)
            # --- L0 elementwise from g0_prev = g0(p): -> h0(p), h0T(p) ---
            a0 = []
            for n in range(NB):
                an = apool.tile([BL, 512], DT.float32, tag="a0", name=f"a0_{p}_{n}")
                nc.scalar.activation(an[:], g0_prev[n][:], GFUNC[n])
                a0.append(an)
            tig0 = spool.tile([BL, H], DT.float32, tag="tig0", name=f"tig0_{p}")
            nc.vector.tensor_mul(tig0[:], a0[0][:], a0[2][:])
            cc0 = cpool.tile([BL, H], DT.float32, tag="c0", name=f"c0_{p}")
            if c0_prev is None:
                nc.vector.tensor_copy(cc0[:], tig0[:])
            else:
                nc.vector.tensor_mul(cc0[:], a0[1][:], c0_prev[:])
                nc.vector.tensor_add(cc0[:], cc0[:], tig0[:])
            c0_prev = cc0
            tch0 = spool.tile([BL, H], DT.float32, tag="tch0", name=f"tch0_{p}")
            nc.scalar.activation(tch0[:], cc0[:], AF.Tanh)
            h0 = spool.tile([BL, H], BF, tag="h0", name=f"h0_{p}")
            nc.vector.tensor_mul(h0[:], a0[3][:], tch0[:])
            h0T = hpool.tile([128, KC * BL], BF, tag="h0T", name=f"h0T_{p}")
            for k in range(KC):
                nc.sync.dma_start(h0T[:, k * BL:(k + 1) * BL],
                                  h0[:, k * 128:(k + 1) * 128], transpose=True)

            # --- L1 elementwise from g1_prev = g1(p-1): -> h1(p-1), h1T(p-1) ---
            if g1_prev is not None:
                a1 = []
                for n in range(NB):
                    an = apool.tile([128, 512], DT.float32, tag="a1", name=f"a1_{p}_{n}")
                    nc.scalar.activation(an[64:128, :], g1_prev[n][64:128, :], GFUNC[n])
                    a1.append(an)
                tig1 = spool.tile([128, H], DT.float32, tag="tig1", name=f"tig1_{p}")
                nc.vector.tensor_mul(tig1[64:128, :], a1[0][64:128, :], a1[2][64:128, :])
                cc1 = cpool.tile([128, H], DT.float32, tag="c1", name=f"c1_{p}")
                if c1_prev is None:
                    nc.vector.tensor_copy(cc1[64:128, :], tig1[64:128, :])
                else:
                    nc.vector.tensor_mul(cc1[64:128, :], a1[1][64:128, :], c1_prev[64:128, :])
                    nc.vector.tensor_add(cc1[64:128, :], cc1[64:128, :], tig1[64:128, :])
                c1_prev = cc1
                tch1 = spool.tile([128, H], DT.float32, tag="tch1", name=f"tch1_{p}")
                nc.scalar.activation(tch1[64:128, :], cc1[64:128, :], AF.Tanh)
                h1 = spool.tile([128, H], BF, tag="h1", name=f"h1_{p}")
                nc.vector.tensor_mul(h1[64:128, :], a1[3][64:128, :], tch1[64:128, :])
                h1T = hpool.tile([128, KC * BL], BF, tag="h1T", name=f"h1T_{p}")
                for k in range(KC):
                    nc.scalar.dma_start(h1T[:, k * BL:(k + 1) * BL],
                                        h1[64:128, k * 128:(k + 1) * 128], transpose=True)
            else:
                h1T = h1T_zero

            # --- matmul wave: A: g0(p+1) + fc(p-1); B: g1(p) ---
            g0_cur = [g0p.tile([BL, 512], DT.float32, tag="g0", name=f"g0_{p}_{n}")
                      for n in range(NB)] if not last else None
            g1_cur = [g1p.tile([128, 512], DT.float32, tag="g1", name=f"g1_{p}_{n}")
                      for n in range(NB)]
            amms = []
            if not last:
                for n in range(NB):
                    amms.append((g0_cur[n][:], ohT[:, (p + 1) * BL:(p + 2) * BL],
                                 u_sb[:, n * 512:(n + 1) * 512], True, False))
                    for k in range(KC):
                        amms.append((g0_cur[n][:], h0T[:, k * BL:(k + 1) * BL],
                                     w_hh0[:, k * G + n * 512:k * G + (n + 1) * 512],
                                     False, k == KC - 1))
            if p >= 1:
                fcps = fcp.tile([BL, V], DT.float32, tag="fc", name=f"fc_{p-1}")
                for k in range(KC):
                    amms.append((fcps[:], h1T[:, k * BL:(k + 1) * BL],
                                 w_fc[:, k * V:(k + 1) * V], k == 0, k == KC - 1))
            else:
                fcps = None
            bmms = []
            for n in range(NB):
                first = True
                if p >= 1:
                    for k in range(KC):
                        bmms.append((g1_cur[n][64:128, :], h1T[:, k * BL:(k + 1) * BL],
                                     w_hh1[:, k * G + n * 512:k * G + (n + 1) * 512],
                                     k == 0, False))
                    first = False
                for k in range(KC):
                    bmms.append((g1_cur[n][64:128, :], h0T[:, k * BL:(k + 1) * BL],
                                 w_ih1[:, k * G + n * 512:k * G + (n + 1) * 512],
                                 first and k == 0, k == KC - 1))
            for i in range(max(len(amms), len(bmms))):
                if i < len(bmms):
                    o, l, r, st, sp = bmms[i]
                    nc.tensor.matmul(o, l, r, start=st, stop=sp, **TPB)
                if i < len(amms):
                    o, l, r, st, sp = amms[i]
                    nc.tensor.matmul(o, l, r, start=st, stop=sp, **TPA)
            for n in range(NB):
                nc.vector.tensor_add(g1_cur[n][64:128, :], g1_cur[n][64:128, :],
                                     bb[64:128, n * 512:(n + 1) * 512])
            if fcps is not None:
                nc.vector.tensor_add(fcps[:], fcps[:], bfb[:])
                fc_tail(fcps, p - 1)

            g0_prev = g0_cur
            g1_prev = g1_cur

        # tail: g1(nsteps-1) -> h1(last), fc(last)
        a1 = []
        for n in range(NB):
            an = apool.tile([128, 512], DT.float32, tag="a1", name=f"a1_tail_{n}")
            nc.scalar.activation(an[64:128, :], g1_prev[n][64:128, :], GFUNC[n])
            a1.append(an)
        tig1 = spool.tile([128, H], DT.float32, tag="tig1", name="tig1_tail")
        nc.vector.tensor_mul(tig1[64:128, :], a1[0][64:128, :], a1[2][64:128, :])
        cc1 = cpool.tile([128, H], DT.float32, tag="c1", name="c1_tail")
        nc.vector.tensor_mul(cc1[64:128, :], a1[1][64:128, :], c1_prev[64:128, :])
        nc.vector.tensor_add(cc1[64:128, :], cc1[64:128, :], tig1[64:128, :])
        tch1 = spool.tile([128, H], DT.float32, tag="tch1", name="tch1_tail")
        nc.scalar.activation(tch1[64:128, :], cc1[64:128, :], AF.Tanh)
        h1 = spool.tile([128, H], BF, tag="h1", name="h1_tail")
        nc.vector.tensor_mul(h1[64:128, :], a1[3][64:128, :], tch1[64:128, :])
        h1T = hpool.tile([128, KC * BL], BF, tag="h1T", name="h1T_tail")
        for k in range(KC):
            nc.scalar.dma_start(h1T[:, k * BL:(k + 1) * BL],
                                h1[64:128, k * 128:(k + 1) * 128], transpose=True)
        fcps = fcp.tile([BL, V], DT.float32, tag="fc", name="fc_tail")
        for k in range(KC):
            nc.tensor.matmul(fcps[:], h1T[:, k * BL:(k + 1) * BL],
                             w_fc[:, k * V:(k + 1) * V],
                             start=(k == 0), stop=(k == KC - 1), **TPA)
        nc.vector.tensor_add(fcps[:], fcps[:], bfb[:])
        fc_tail(fcps, nsteps - 1)

        # probs phase
        PC = 16
        for c0 in range(0, nsteps, PC):
            cw = min(PC, nsteps - c0)
            exc = spool.tile([BL, PC * V], DT.float32, tag="exc", name=f"exc_{c0}")
            nc.scalar.activation(exc[:, :cw * V], lgbuf[:, c0 * V:(c0 + cw) * V], AF.Exp)
            sec = spool.tile([BL, PC], DT.float32, tag="sec", name=f"sec_{c0}")
            nc.vector.reduce_sum(sec[:, :cw],
                                 exc[:, :cw * V].rearrange("p (t v) -> p t v", v=V),
                                 axis=mybir.AxisListType.X)
            rcc = spool.tile([BL, PC], DT.float32, tag="rcc", name=f"rcc_{c0}")
            nc.vector.reciprocal(rcc[:, :cw], sec[:, :cw])
            prc = spool.tile([BL, PC * V], DT.float32, tag="prc", name=f"prc_{c0}")
            for j in range(cw):
                nc.vector.tensor_scalar(prc[:, j * V:(j + 1) * V],
                                        exc[:, j * V:(j + 1) * V],
                                        rcc[:, j:j + 1], None, op0=ALU.mult)
            nc.sync.dma_start(o_probs[:, c0:c0 + cw, :], prc[:, :cw * V])

        slsum = spool.tile([BL, 1], DT.float32, tag="slsum")
        nc.vector.reduce_sum(slsum[:], nllb[:, :nsteps], axis=mybir.AxisListType.X,
                             negate=True)
        sl = spool.tile([BL, 1], DT.float32, tag="sl")
        nc.vector.tensor_mul(sl[:], slsum[:], invlen[:])
        nc.sync.dma_start(o_sloss[:], sl[:])

    return nc


USE_V2 = os.environ.get("KV2", "0") == "1"


def build(t_steps=T):
    return build_graph_v2(t_steps) if USE_V2 else build_graph(t_steps)


def _prep_maps(x, emb_table, W_ih0, W_hh0, b_ih0, b_hh0,
               W_ih1, W_hh1, b_ih1, b_hh1, W_fc, b_fc, t_steps=T):
    """Host-side shard + layout prep (slicing / transposes / dtype casts only)."""
    f32 = np.float32
    x = np.asarray(x)
    xi = x.astype(np.int32)
    c = lambda a: np.ascontiguousarray(np.asarray(a), dtype=f32)
    shared = {
        "embT": c(np.asarray(emb_table).T),
        "emb_tab": c(emb_table),
        "wih0T": c(np.asarray(W_ih0).T),
        "wh0T": c(np.asarray(W_hh0).T),
        "wi1T": c(np.asarray(W_ih1).T),
        "wh1T": c(np.asarray(W_hh1).T),
        "wfcT": c(np.asarray(W_fc).T),
        "b_i0": c(b_ih0).reshape(1, -1),
        "b_h0": c(b_hh0).reshape(1, -1),
        "b_i1": c(b_ih1).reshape(1, -1),
        "b_h1": c(b_hh1).reshape(1, -1),
        "b_fc": c(b_fc).reshape(1, -1),
    }
    in_maps = []
    for i in range(NCORES):
        xs = xi[i * BL:(i + 1) * BL, :t_steps]
        m = dict(shared)
        m["x_i"] = np.ascontiguousarray(xs)
        m["x_row"] = np.ascontiguousarray(xs[:, :t_steps - 1].T.reshape(1, -1))
        in_maps.append(m)
    return in_maps


_NC_CACHE = {}


def kernel(x, emb_table, W_ih0, W_hh0, b_ih0, b_hh0,
           W_ih1, W_hh1, b_ih1, b_hh1, W_fc, b_fc):
    if T not in _NC_CACHE:
        nc_ = build(T)
        nc_.finalize()
        _NC_CACHE[T] = nc_
    nc = _NC_CACHE[T]
    in_maps = _prep_maps(x, emb_table, W_ih0, W_hh0, b_ih0, b_hh0,
                         W_ih1, W_hh1, b_ih1, b_hh1, W_fc, b_fc)
    res = run_bass_kernel_spmd(nc, in_maps, core_ids=list(range(NCORES)))
    outs = res.results
    probs = np.concatenate([o["probs"].reshape(BL, T - 1, V) for o in outs], axis=0)
    emb = np.concatenate([o["emb"].reshape(BL, T, E) for o in outs], axis=0)
    sloss = np.concatenate([o["sloss"].reshape(BL) for o in outs], axis=0)
    mean = np.float32(sloss.mean())
    return probs, emb, sloss, np.asarray(mean)
